# revision 1
# baseline (speedup 1.0000x reference)
"""Trainium2 Bass kernel for nn_BiEncoder_63024350101542 (segment_reduce).

Computes, per batch row b of vector_all [B=64, L=512, D=1024]:
    mask[b,j] = (j > first_idx(ids[b]==1)) & (j < first_idx(ids[b]==2))
    span_max  = max over masked rows (fallback: CLS row 0 when mask empty)
    out[b]    = cls + mu * span_max

Sharding: pure data parallelism over the batch dim — 8 batches per
NeuronCore across 8 cores. Each core streams its 16 MiB shard of
vector_all once (memory-bound), doing the masked max on-chip.

Note: every PE (transpose) instruction must carry at most one semaphore
wait — walrus rejects matmuls with multiple embedded waits. All PE
inputs are therefore produced by the vector engine (single DVE sem).
"""

import os
import sys

import numpy as np

for _p in ("/root/.axon_site/_ro/trn_rl_repo", "/opt/trn_rl_repo"):
    if _p not in sys.path and os.path.isdir(_p):
        sys.path.append(_p)

import concourse.bacc as bacc
import concourse.bass as bass
import concourse.mybir as mybir
import concourse.tile as tile
from concourse.bass_utils import run_bass_kernel_spmd

F32 = mybir.dt.float32
BF16 = mybir.dt.bfloat16
I32 = mybir.dt.int32
X = mybir.AxisListType.X
Alu = mybir.AluOpType
Act = mybir.ActivationFunctionType

B, L, D = 64, 512, 1024
NCORES = 8
BPC = B // NCORES          # batches per core
KL = L // 128              # L-tiles per batch (4)
JD = D // 128              # d-blocks (8)
BIG = 1.0e30


def build_bass():
    nc = bacc.Bacc("TRN2", target_bir_lowering=False, debug=False)

    va = nc.dram_tensor("vector_all", [BPC, L, D], F32, kind="ExternalInput").ap()
    ids = nc.dram_tensor("ids", [BPC, L], I32, kind="ExternalInput").ap()
    mu = nc.dram_tensor("mu", [128, 1], F32, kind="ExternalInput").ap()
    iota = nc.dram_tensor("iota", [BPC, L], F32, kind="ExternalInput").ap()
    iotap = nc.dram_tensor("iotap", [128, KL], F32, kind="ExternalInput").ap()
    ident = nc.dram_tensor("identity", [128, 128], F32, kind="ExternalInput").ap()
    out = nc.dram_tensor("out", [BPC, D], F32, kind="ExternalOutput").ap()

    with tile.TileContext(nc) as tc:
        with (
            tc.tile_pool(name="persist", bufs=1) as pp,
            tc.tile_pool(name="xin", bufs=4) as xpool,
            tc.tile_pool(name="masked", bufs=4) as mpool,
            tc.tile_pool(name="red", bufs=2) as rpool,
            tc.tile_pool(name="vout", bufs=2) as vpool,
            tc.tile_pool(name="tr", bufs=4, space="PSUM") as ppool,
            tc.tile_pool(name="smallp", bufs=1, space="PSUM") as spsum,
        ):
            # ---- constants / inputs for the mask stage (POOL ring) ----
            ids_sb = pp.tile([BPC, L], I32)
            nc.gpsimd.dma_start(out=ids_sb[:], in_=ids)
            iota_sb = pp.tile([BPC, L], F32)
            nc.gpsimd.dma_start(out=iota_sb[:], in_=iota)
            ident_sb = pp.tile([128, 128], F32)
            nc.gpsimd.dma_start(out=ident_sb[:], in_=ident)
            mu_col = pp.tile([128, 1], F32)
            nc.gpsimd.dma_start(out=mu_col[:], in_=mu)
            iotap_sb = pp.tile([128, KL], F32)
            nc.gpsimd.dma_start(out=iotap_sb[:], in_=iotap)
            ones_row = pp.tile([1, 128], F32)
            nc.vector.memset(ones_row[:], 1.0)
            # CLS rows in vec layout: cls_f[m, b, i] = vector_all[b, 0, 32m+i]
            cls_f = pp.tile([32, BPC, 32], F32)
            nc.gpsimd.dma_start(
                out=cls_f[:],
                in_=va[:, 0, :].rearrange("b (m i) -> m b i", i=32),
            )

            # ---- queue the big streaming loads (SP / POOL rings) ----
            xs = []
            for b in range(BPC):
                x = xpool.tile([128, KL, D], F32, tag="x")
                dma_eng = nc.sync if b % 2 == 0 else nc.gpsimd
                # 16 KiB contiguous per partition: l = 4p + k
                dma_eng.dma_start(
                    out=x[:], in_=va[b].rearrange("(p k) d -> p k d", k=KL)
                )
                xs.append(x)

            # ---- mask stage ----
            # fs[:, 0] = first1, fs[:, 1] = first2, fs[:, 2] = has_span
            fs = pp.tile([BPC, 3], F32)

            def first_idx(marker: int, col: int):
                t = pp.tile([BPC, L], F32, tag=f"t{marker}")
                nc.vector.memset(t[:], float(L))
                ism = pp.tile([BPC, L], I32, tag=f"is{marker}")
                nc.vector.tensor_scalar(
                    out=ism[:], in0=ids_sb[:], scalar1=marker, scalar2=None,
                    op0=Alu.is_equal,
                )
                nc.vector.copy_predicated(t[:], ism[:], iota_sb[:])
                nc.vector.tensor_reduce(
                    fs[:, col : col + 1], t[:], axis=X, op=Alu.min
                )

            first_idx(1, 0)
            first_idx(2, 1)
            # has_span = (first1 + 1 < first2)
            f1p1 = pp.tile([BPC, 1], F32)
            nc.vector.tensor_scalar_add(f1p1[:], fs[:, 0:1], 1.0)
            nc.vector.tensor_tensor(
                out=fs[:, 2:3], in0=f1p1[:], in1=fs[:, 1:2], op=Alu.is_lt
            )

            # transpose each column of fs to a [1, BPC] row at partition 0
            fsT = pp.tile([1, 3, BPC], F32)
            for c in range(3):
                rT = spsum.tile([1, BPC], F32, tag="small")
                nc.tensor.transpose(
                    rT[:], fs[:, c : c + 1], ident_sb[0:BPC, 0:BPC]
                )
                nc.vector.tensor_copy(fsT[:, c, :], rT[:])

            # broadcast first1/first2 across partitions: [128, 2, BPC]
            f12r_ps = spsum.tile([128, 2, BPC], F32, tag="small")
            nc.tensor.matmul(f12r_ps[:], lhsT=ones_row[:], rhs=fsT[:, 0:2, :])
            f1r_ps = f12r_ps[:, 0, :]
            f2r_ps = f12r_ps[:, 1, :]

            # maskT[p, k*BPC+b] = (4p+k > first1[b]) & (4p+k < first2[b])
            maskT = pp.tile([128, KL * BPC], F32)
            for k in range(KL):
                ga = pp.tile([128, BPC], F32, tag="ga")
                nc.vector.tensor_scalar(
                    out=ga[:], in0=f1r_ps, scalar1=iotap_sb[:, k : k + 1],
                    scalar2=None, op0=Alu.is_lt,
                )
                gb = pp.tile([128, BPC], F32, tag="gb")
                nc.vector.tensor_scalar(
                    out=gb[:], in0=f2r_ps, scalar1=iotap_sb[:, k : k + 1],
                    scalar2=None, op0=Alu.is_gt,
                )
                nc.vector.tensor_mul(maskT[:, bass.ts(k, BPC)], ga[:], gb[:])
            # row 0 (l = 0: p=0, k=0) contributes CLS exactly when span empty
            nc.vector.tensor_scalar(
                out=maskT[0:1, 0:BPC], in0=fsT[:, 2, :], scalar1=-1.0, scalar2=1.0,
                op0=Alu.mult, op1=Alu.add,
            )
            biasT = pp.tile([128, KL * BPC], F32)
            nc.vector.tensor_scalar(
                out=biasT[:], in0=maskT[:], scalar1=BIG, scalar2=BIG,
                op0=Alu.mult, op1=Alu.subtract,
            )

            # vec accumulator: fin_all[m, b, i] = vec_b[32m + i]
            fin_all = pp.tile([32, BPC, 32], F32)

            # ---- main streaming loop ----
            for b in range(BPC):
                x = xs[b]

                # masked copy on ScalarE: m*x + (m-1)*BIG
                xm = mpool.tile([128, KL, D], F32, tag="xm")
                for k in range(KL):
                    col = k * BPC + b
                    nc.scalar.activation(
                        xm[:, k, :], x[:, k, :], Act.Identity,
                        bias=biasT[:, col : col + 1],
                        scale=maskT[:, col : col + 1],
                    )

                # max over the 4 L-tiles -> r [128, D]
                t01 = rpool.tile([128, D], F32, tag="t01")
                nc.vector.tensor_max(t01[:], xm[:, 0, :], xm[:, 1, :])
                t23 = rpool.tile([128, D], F32, tag="t23")
                nc.vector.tensor_max(t23[:], xm[:, 2, :], xm[:, 3, :])
                r = rpool.tile([128, D], F32, tag="r")
                nc.vector.tensor_max(r[:], t01[:], t23[:])

                # cross-partition max, stage 1: 32x32 transpose-fused reduce.
                # s1[32a+i, m] = max over partition group a of column 32m+i
                s1 = vpool.tile([128, 32], F32, tag="s1")
                nc.vector.tensor_reduce(
                    s1[:], r[:].rearrange("p (m c) -> p m c", c=32),
                    axis=X, op=Alu.max, apply_transpose=True,
                )
                # stage 2: transpose s1, then max the 4 partition groups
                s1T = ppool.tile([32, 128], F32, tag="s1T")
                nc.tensor.transpose(s1T[:], s1[:], ident_sb[:])
                nc.vector.tensor_reduce(
                    fin_all[:, b, :],
                    s1T[:].rearrange("p (a i) -> p i a", a=4),
                    axis=X, op=Alu.max,
                )

            # ---- store: out = cls + mu*vec, in [32, b, 32] layout ----
            oT = vpool.tile([32, BPC, 32], F32, tag="oT")
            nc.vector.scalar_tensor_tensor(
                out=oT[:], in0=fin_all[:], scalar=mu_col[0:32, 0:1],
                in1=cls_f[:], op0=Alu.mult, op1=Alu.add,
            )
            nc.sync.dma_start(
                out=out.rearrange("b (m i) -> m b i", i=32), in_=oT[:]
            )

    nc.compile()
    return nc


def make_const_inputs():
    iota = np.broadcast_to(
        np.arange(L, dtype=np.float32)[None, :], (BPC, L)
    ).copy()
    # iotap[p, k] = l = 4p + k (row index held by partition p, col group k)
    iotap = (
        np.arange(128, dtype=np.float32)[:, None] * KL
        + np.arange(KL, dtype=np.float32)[None, :]
    )
    ident = np.eye(128, dtype=np.float32)
    return iota, iotap, ident


def make_in_maps(vector_all, ids, mu):
    va = np.ascontiguousarray(np.asarray(vector_all, dtype=np.float32))
    ids = np.ascontiguousarray(np.asarray(ids, dtype=np.int32))
    mu_col = np.full((128, 1), np.asarray(mu, dtype=np.float32).reshape(-1)[0],
                     dtype=np.float32)
    iota, iotap, ident = make_const_inputs()
    in_maps = []
    for c in range(NCORES):
        in_maps.append(
            {
                "vector_all": va[c * BPC : (c + 1) * BPC],
                "ids": ids[c * BPC : (c + 1) * BPC],
                "mu": mu_col,
                "iota": iota,
                "iotap": iotap,
                "identity": ident,
            }
        )
    return in_maps


def run(vector_all, ids, mu, trace=False):
    """Returns (out [B, D] f32, BassKernelResults)."""
    nc = build_bass()
    in_maps = make_in_maps(vector_all, ids, mu)
    res = run_bass_kernel_spmd(nc, in_maps, list(range(NCORES)), trace=trace)
    out = np.concatenate(
        [res.results[c]["out"] for c in range(NCORES)], axis=0
    ).astype(np.float32)
    return out, res


def kernel(**inputs) -> np.ndarray:
    out, _ = run(inputs["vector_all"], inputs["ids"], inputs["mu"])
    return out



# revision 2
# speedup vs baseline: 1.2193x; 1.2193x over previous
"""Trainium2 Bass kernel for nn_BiEncoder_63024350101542 (segment_reduce).

Computes, per batch row b of vector_all [B=64, L=512, D=1024]:
    mask[b,j] = (j > first_idx(ids[b]==1)) & (j < first_idx(ids[b]==2))
    span_max  = max over masked rows (fallback: CLS row 0 when mask empty)
    out[b]    = cls + mu * span_max

Strategy (v6): stream each batch in the chunk layout l = 128c + p
(chunk c on the free axis, row-within-chunk p on partitions). The span
[first1+1, first2) then covers, per chunk, a contiguous partition range
[L_c, T_c) — so masking collapses to one per-partition bias column
(+0 inside, -BIG outside) per (batch, chunk), applied *inside* the max
fold: u0 = Identity(x_c0 + bias0) on Act, then three
scalar_tensor_tensor (add bias, max accumulate) links on DVE, then a
transpose-fused cross-partition reduce. No separate mask stage, no
memsets. Empty spans override chunk 0's window to [0, 1), selecting
exactly the CLS row. All-f32 => bit-exact outputs.

DMA streams the full tensor (static APs only: dynamic/indirect DMA
faults on this runtime), spread over SWDGE + both HWDGE queues.
Sharding: pure data parallelism over batch — 8 batches per core.
"""

import os
import sys

import numpy as np

for _p in ("/root/.axon_site/_ro/trn_rl_repo", "/opt/trn_rl_repo"):
    if _p not in sys.path and os.path.isdir(_p):
        sys.path.append(_p)

import concourse.bacc as bacc
import concourse.bass as bass
import concourse.mybir as mybir
import concourse.tile as tile
from concourse.bass_utils import run_bass_kernel_spmd

F32 = mybir.dt.float32
I32 = mybir.dt.int32
X = mybir.AxisListType.X
Alu = mybir.AluOpType
Act = mybir.ActivationFunctionType

B, L, D = 64, 512, 1024
NCORES = 8
BPC = B // NCORES
G = 4                      # chunks per batch (L / 128)
W = 128                    # rows per chunk
BIG = 1.0e30


def build_bass():
    nc = bacc.Bacc("TRN2", target_bir_lowering=False, debug=False)

    va = nc.dram_tensor("vector_all", [BPC, L, D], F32, kind="ExternalInput").ap()
    ids = nc.dram_tensor("ids", [BPC, L], I32, kind="ExternalInput").ap()
    mu = nc.dram_tensor("mu", [128, 1], F32, kind="ExternalInput").ap()
    out = nc.dram_tensor("out", [BPC, D], F32, kind="ExternalOutput").ap()

    with tile.TileContext(nc) as tc:
        with (
            tc.tile_pool(name="persist", bufs=1) as pp,
            tc.tile_pool(name="xin", bufs=3) as xpool,
            tc.tile_pool(name="fold", bufs=3) as fpool,
            tc.tile_pool(name="ps", bufs=4, space="PSUM") as ppool,
            tc.tile_pool(name="pss", bufs=2, space="PSUM") as spsum,
        ):
            # ---- ids first (tiny, on sync); constants built on-device --
            ids_sb = pp.tile([BPC, L], I32)
            nc.scalar.dma_start(out=ids_sb[:], in_=ids)
            mu_col = pp.tile([128, 1], F32)
            nc.scalar.dma_start(out=mu_col[:], in_=mu)

            iota_i = pp.tile([BPC, L], I32)
            nc.gpsimd.iota(iota_i[:], pattern=[[1, L]], channel_multiplier=0)
            iota_sb = pp.tile([BPC, L], F32)
            nc.vector.tensor_copy(iota_sb[:], iota_i[:])
            prow_i = pp.tile([128, 1], I32)
            nc.gpsimd.iota(prow_i[:], pattern=[[0, 1]], channel_multiplier=1)
            piota_sb = pp.tile([128, 1], F32)
            nc.vector.tensor_copy(piota_sb[:], prow_i[:])
            qrow_i = pp.tile([128, 128], I32)
            nc.gpsimd.iota(qrow_i[:], pattern=[[1, 128]], channel_multiplier=0)
            ident_i = pp.tile([128, 128], I32)
            nc.vector.tensor_tensor(
                out=ident_i[:], in0=qrow_i[:],
                in1=prow_i[:].to_broadcast([128, 128]), op=Alu.is_equal,
            )
            ident_sb = pp.tile([128, 128], F32)
            nc.vector.tensor_copy(ident_sb[:], ident_i[:])
            ones_row = pp.tile([1, 128], F32)
            nc.vector.memset(ones_row[:], 1.0)
            # CLS rows: cls_f[m, b, i] = vector_all[b, 0, 32m+i]
            cls_f = pp.tile([32, BPC, 32], F32)
            nc.gpsimd.dma_start(
                out=cls_f[:],
                in_=va[:, 0, :].rearrange("b (m i) -> m b i", i=32),
            )

            # ---- per-batch streaming loads on ONE queue: batches
            # arrive sequentially, so batch b's fold starts ~6us after
            # batch b-1's ----
            xs = []
            for b in range(BPC - 1):
                x = xpool.tile([128, G, D], F32, tag="x")
                nc.sync.dma_start(
                    out=x[:], in_=va[b].rearrange("(c p) d -> p c d", p=W)
                )
                xs.append(x)
            xlast = []
            for c in range(G):
                xc = xpool.tile([128, D], F32, tag=f"xl{c}")
                nc.sync.dma_start(
                    out=xc[:], in_=va[BPC - 1, c * W : (c + 1) * W, :]
                )
                xlast.append(xc)

            # ---- mask stage: first1, first2, has_span ----
            fs = pp.tile([BPC, 3], F32)

            def first_idx(marker: int, col: int):
                t = pp.tile([BPC, L], F32, tag=f"t{marker}")
                nc.vector.memset(t[:], float(L))
                ism = pp.tile([BPC, L], I32, tag=f"is{marker}")
                nc.vector.tensor_scalar(
                    out=ism[:], in0=ids_sb[:], scalar1=marker, scalar2=None,
                    op0=Alu.is_equal,
                )
                nc.vector.copy_predicated(t[:], ism[:], iota_sb[:])
                nc.vector.tensor_reduce(
                    fs[:, col : col + 1], t[:], axis=X, op=Alu.min
                )

            first_idx(1, 0)
            first_idx(2, 1)
            f1p1 = pp.tile([BPC, 1], F32)
            nc.vector.tensor_scalar_add(f1p1[:], fs[:, 0:1], 1.0)
            nc.vector.tensor_tensor(
                out=fs[:, 2:3], in0=f1p1[:], in1=fs[:, 1:2], op=Alu.is_lt
            )
            hs = fs[:, 2:3]

            # ---- per-(batch, chunk) thresholds in fs-space [BPC, G] ----
            # L_c = clamp(first1+1-128c, 0, 128); T_c = clamp(first2-128c,..)
            # empty span: L_0 = 0, T_0 = 1  (selects row 0 = CLS)
            Lg = pp.tile([BPC, G], F32)
            Tg = pp.tile([BPC, G], F32)
            for g in range(G):
                nc.vector.tensor_scalar(
                    out=Lg[:, g : g + 1], in0=f1p1[:],
                    scalar1=float(-W * g), scalar2=0.0,
                    op0=Alu.add, op1=Alu.max,
                )
                nc.vector.tensor_scalar(
                    out=Tg[:, g : g + 1], in0=fs[:, 1:2],
                    scalar1=float(-W * g), scalar2=0.0,
                    op0=Alu.add, op1=Alu.max,
                )
            nc.vector.tensor_scalar(
                out=Lg[:], in0=Lg[:], scalar1=float(W), scalar2=None,
                op0=Alu.min,
            )
            nc.vector.tensor_scalar(
                out=Tg[:], in0=Tg[:], scalar1=float(W), scalar2=None,
                op0=Alu.min,
            )
            # empty-span overrides on chunk 0
            nc.vector.tensor_tensor(
                out=Lg[:, 0:1], in0=Lg[:, 0:1], in1=hs, op=Alu.mult
            )
            # T0 = hs*(T0-1) + 1
            t0m1 = pp.tile([BPC, 1], F32)
            nc.vector.tensor_scalar(
                out=t0m1[:], in0=Tg[:, 0:1], scalar1=1.0, scalar2=None,
                op0=Alu.subtract,
            )
            nc.vector.tensor_tensor(
                out=Tg[:, 0:1], in0=t0m1[:], in1=hs, op=Alu.mult
            )
            nc.vector.tensor_scalar_add(Tg[:, 0:1], Tg[:, 0:1], 1.0)

            # ---- transpose L|T to partition-0 rows, broadcast, biases ----
            sc = pp.tile([BPC, 2 * G], F32)
            nc.vector.tensor_copy(sc[:, 0:G], Lg[:])
            nc.vector.tensor_copy(sc[:, G : 2 * G], Tg[:])
            scT = pp.tile([1, 2 * G, BPC], F32)
            for c in range(2 * G):
                rT = spsum.tile([1, BPC], F32, tag="scT")
                nc.tensor.transpose(
                    rT[:], sc[:, c : c + 1], ident_sb[0:BPC, 0:BPC]
                )
                nc.vector.tensor_copy(scT[:, c, :], rT[:])
            ltr_ps = spsum.tile([128, 2 * G, BPC], F32, tag="ltr")
            nc.tensor.matmul(ltr_ps[:], lhsT=ones_row[:], rhs=scT[:])
            # bias[p, c, b] = 0 if L<=p<T else -BIG
            pge = pp.tile([128, G, BPC], F32)
            nc.vector.tensor_tensor(
                out=pge[:], in0=piota_sb[:].to_broadcast([128, G, BPC]),
                in1=ltr_ps[:, 0:G, :], op=Alu.is_ge,
            )
            plt = pp.tile([128, G, BPC], F32)
            nc.vector.tensor_tensor(
                out=plt[:], in0=piota_sb[:].to_broadcast([128, G, BPC]),
                in1=ltr_ps[:, G : 2 * G, :], op=Alu.is_lt,
            )
            bias = pp.tile([128, G, BPC], F32)
            nc.vector.tensor_tensor(
                out=bias[:], in0=pge[:], in1=plt[:], op=Alu.mult
            )
            nc.vector.tensor_scalar(
                out=bias[:], in0=bias[:], scalar1=BIG, scalar2=BIG,
                op0=Alu.mult, op1=Alu.subtract,
            )

            # ---- per-batch masked max chain + cross-partition reduce ----
            fin_all = pp.tile([32, BPC, 32], F32)
            for b in range(BPC):
                if b < BPC - 1:
                    xk = [xs[b][:, c, :] for c in range(G)]
                else:
                    xk = [xlast[c][:] for c in range(G)]
                u0 = fpool.tile([128, D], F32, tag="u0")
                nc.scalar.activation(
                    u0[:], xk[0], Act.Identity,
                    bias=bias[:, 0, b : b + 1], scale=1.0,
                )
                m1 = fpool.tile([128, D], F32, tag="m1")
                nc.scalar.activation(
                    m1[:], xk[1], Act.Identity,
                    bias=bias[:, 1, b : b + 1], scale=1.0,
                )
                u1 = fpool.tile([128, D], F32, tag="u1")
                nc.vector.tensor_max(u1[:], u0[:], m1[:])
                u2 = fpool.tile([128, D], F32, tag="u2")
                nc.vector.scalar_tensor_tensor(
                    out=u2[:], in0=xk[2], scalar=bias[:, 2, b : b + 1],
                    op0=Alu.add, op1=Alu.max, in1=u1[:],
                )
                u3 = fpool.tile([128, D], F32, tag="u3")
                nc.vector.scalar_tensor_tensor(
                    out=u3[:], in0=xk[3], scalar=bias[:, 3, b : b + 1],
                    op0=Alu.add, op1=Alu.max, in1=u2[:],
                )
                s1 = fpool.tile([128, 32], F32, tag="s1")
                nc.vector.tensor_reduce(
                    s1[:], u3[:].rearrange("p (m c) -> p m c", c=32),
                    axis=X, op=Alu.max, apply_transpose=True,
                )
                s1T = ppool.tile([32, 128], F32, tag="s1T")
                nc.tensor.transpose(s1T[:], s1[:], ident_sb[:])
                nc.vector.tensor_reduce(
                    fin_all[:, b, :],
                    s1T[:].rearrange("p (a i) -> p i a", a=4),
                    axis=X, op=Alu.max,
                )

            # ---- out = cls + mu * vec ----
            oT = pp.tile([32, BPC, 32], F32)
            nc.vector.scalar_tensor_tensor(
                out=oT[:], in0=fin_all[:], scalar=mu_col[0:32, 0:1],
                in1=cls_f[:], op0=Alu.mult, op1=Alu.add,
            )
            nc.sync.dma_start(
                out=out.rearrange("b (m i) -> m b i", i=32), in_=oT[:]
            )

    nc.compile()
    return nc


def make_in_maps(vector_all, ids, mu):
    va = np.ascontiguousarray(np.asarray(vector_all, dtype=np.float32))
    ids = np.ascontiguousarray(np.asarray(ids, dtype=np.int32))
    mu_col = np.full(
        (128, 1), np.asarray(mu, dtype=np.float32).reshape(-1)[0],
        dtype=np.float32,
    )
    in_maps = []
    for cix in range(NCORES):
        in_maps.append(
            {
                "vector_all": va[cix * BPC : (cix + 1) * BPC],
                "ids": ids[cix * BPC : (cix + 1) * BPC],
                "mu": mu_col,
            }
        )
    return in_maps


def run(vector_all, ids, mu, trace=False):
    nc = build_bass()
    in_maps = make_in_maps(vector_all, ids, mu)
    res = run_bass_kernel_spmd(nc, in_maps, list(range(NCORES)), trace=trace)
    out = np.concatenate(
        [res.results[c]["out"] for c in range(NCORES)], axis=0
    ).astype(np.float32)
    return out, res


def kernel(**inputs) -> np.ndarray:
    out, _ = run(inputs["vector_all"], inputs["ids"], inputs["mu"])
    return out
